# revision 6
# baseline (speedup 1.0000x reference)
"""Chamfer-distance kernel for TRN2 (8 NeuronCores, SPMD).

Math: the reference weights w are nonzero ONLY for points with
time_indice == 1 (m of N points).  So of the NxN distance matrix we only
need row-mins for the m selected rows (dist1) and col-mins for the m
selected columns (dist2) -- each an (m x N) problem, min over N.

Each (m x N) pass is computed as a K=4 matmul:
    C[i, j] = sq[j] - 2 * dot(sel_i, pts_j)
with lhsT rows 0..2 = -2*sel coords, row 3 = ones, and rhs rows 0..2 =
pts coords, row 3 = |pts|^2.  The per-row constant sq[i] of the selected
point is added on the host after the global min.

Perf structure (per 128-row tile, 2048 columns on each core):
  * the 4 512-col chunk matmuls are packed into 4 distinct PE row-groups
    via tile_position (K=4 only occupies 4 of 128 PE rows) and run
    concurrently; the two chunks of the PSUM hi half are issued FIRST so
    the Scalar-engine copy can start at the half-tile point;
  * PSUM is one 4-bank [128, 2048] tile (double-buffered); the Scalar
    engine copies the hi half to SBUF while the Vector engine runs a
    tensor_tensor_reduce (min, min) that folds the PSUM lo half against
    the copy while row-min-reducing -- 2 PSUM elements drained per DVE
    cycle, which is the architectural ceiling (DVE and ACT have one
    32-bit PSUM read port each; GPSIMD has none, DMA is far slower).
  * All inputs for one PE row-group (lhsA | lhsB | rhs chunks) are packed
    in ONE dram tensor so the whole input load is 4 DMA_DIRECT2D issues
    split over both HWDGE queues (the per-issue cost is ~750ns, so the
    baseline's 16 input DMAs wasted ~5us of startup).

Sharding: the N search points are split 2048-per-core across 8 cores
(same lhsT everywhere); each core returns per-row partial mins, the host
takes the elementwise min across cores and does the tiny O(m) tail.
"""

import numpy as np

import concourse.bass as bass
import concourse.mybir as mybir
import concourse.tile as tile
from concourse import bacc
from concourse import dve_ops as _dvo
from concourse.bass_utils import run_bass_kernel_spmd
from concourse.dve_spec import Spec, Src0, Src1, C0, AluOp, minn, lower
from concourse.dve_spec import _has_src1 as _has_src1
from concourse.dve_uop import DveOpSpec


def _make_min2():
    """Register a custom DVE op: out = min(in0, in1), accum_out = row-min."""
    name = "MIN2_REDUCE_ANT"
    for o in _dvo.OPS:
        if o.name == name:
            return o

    def _ref(in0, in1, s0, s1, imm2):
        b = np.minimum(in0, in1).astype(np.float32)
        seed = np.asarray(s0, np.float32).reshape(-1, 1)
        acc = np.minimum(b.reshape(b.shape[0], -1).min(axis=-1, keepdims=True), seed)
        return b, acc

    spec = Spec(body=minn(Src0, Src1), accum=AluOp.MIN, accum_init=C0,
                reference=_ref)
    op = _dvo.DveOp(name, spec, subdim=False, uops_sha={})
    _dvo.OPS.append(op)
    _dvo.CUSTOM_DVE_SPECS[name] = spec
    _dvo._SUB_OPCODE_FOR_NAME[name] = _dvo._CUSTOM_DVE_ROW_BASE + len(_dvo.OPS) - 1
    for ver in ("v3", "v4"):
        ds = DveOpSpec(name=name, opcode=_dvo.get_dve_sub_opcode(name),
                       uops=lower(spec, ver=ver), rd1_en=_has_src1(spec))
        op.uops_sha[ver] = ds.sha(ver)
    return op


_MIN2 = _make_min2()

N_CORES = 8
N_POINTS = 16384
NSHARD = N_POINTS // N_CORES  # 2048 search points per core
FREE = 512                    # matmul moving free dim (one PSUM bank of fp32)
NCC = NSHARD // FREE          # 4 column chunks per row tile

# dtype used for the matmul operands: float32r streams faster than fp32's
# LOW_HIGH dual pass at slightly reduced internal precision.
MM_DT = "float32r"
REDUCER = "min2"   # "min2" = custom DVE op, "ttr" = builtin tensor_tensor_reduce
# PE row-group g computes column chunk CHUNK_OF_GROUP[g].  The hi-half
# chunks (2, 3) go to groups 0/1 so their input DMAs land first and the
# scalar-engine copy of the hi half can start at the half-tile point.
CHUNK_OF_GROUP = (2, 3, 0, 1)

_CACHE = {}


def _build(n_rt):
    """Build + compile the SPMD Bass program for n_rt row-tiles of 128."""
    f32 = mybir.dt.float32
    mdt = getattr(mybir.dt, MM_DT)
    mpad = n_rt * 128
    half = NSHARD // 2
    # one packed input tensor per PE row-group:
    # [lhsA mpad | lhsB mpad | rhsA chunk FREE | rhsB chunk FREE]
    width = 2 * mpad + 2 * FREE

    nc = bacc.Bacc("TRN2", target_bir_lowering=False, debug=False,
                   num_devices=N_CORES, enable_partition_id=False)
    inps = [nc.dram_tensor(f"inp{g}", [4, width], mdt, kind="ExternalInput").ap()
            for g in range(4)]
    outT = nc.dram_tensor("out", [128, 2 * n_rt], f32, kind="ExternalOutput").ap()

    with tile.TileContext(nc) as tc:
        with (
            tc.tile_pool(name="inp", bufs=1) as inp,
            tc.tile_pool(name="res", bufs=1) as res,
            tc.tile_pool(name="cpy", bufs=3) as cpy,
            tc.tile_pool(name="scr", bufs=2) as scr,
            tc.tile_pool(name="ps", bufs=2, space="PSUM") as ps,
        ):
            # One [128, width] SBUF tile; row-group g's data lives on
            # partitions 32g..32g+3.  One DMA per group, alternating HWDGE
            # queues so the issues overlap; groups 0/1 (hi chunks) first.
            it = inp.tile([128, width], mdt, tag="it")
            for g in range(4):
                q = nc.sync if g % 2 == 0 else nc.scalar
                q.dma_start(out=it[32 * g:32 * g + 4, :], in_=inps[g])

            mins = res.tile([128, 2 * n_rt], f32, tag="mins")

            for p in range(2):  # 0 = pass A (dist1), 1 = pass B (dist2)
                lhs_off = p * mpad
                rhs_off = 2 * mpad + p * FREE
                for rt in range(n_rt):
                    pt = ps.tile([128, NSHARD], f32, tag="pt")
                    for g in range(4):
                        cc = CHUNK_OF_GROUP[g]
                        gp = slice(32 * g, 32 * g + 4)
                        nc.tensor.matmul(
                            pt[:, bass.ts(cc, FREE)],
                            it[gp, lhs_off + 128 * rt:lhs_off + 128 * (rt + 1)],
                            it[gp, rhs_off:rhs_off + FREE],
                            start=True, stop=True,
                            tile_position=(32 * g, 0),
                        )
                    # split the row-min: ACT copies the hi half to SBUF, DVE
                    # min-combines the lo PSUM half with it while reducing.
                    cp = cpy.tile([128, half], f32, tag="cp")
                    nc.scalar.copy(out=cp[:], in_=pt[:, half:])
                    sc = scr.tile([128, half], f32, tag="sc")
                    acc = mins[:, p * n_rt + rt:p * n_rt + rt + 1]
                    if REDUCER == "ttr":
                        nc.vector.tensor_tensor_reduce(
                            out=sc[:], in0=pt[:, 0:half], in1=cp[:],
                            scale=1.0, scalar=3.0e38,
                            op0=mybir.AluOpType.min, op1=mybir.AluOpType.min,
                            accum_out=acc,
                        )
                    else:
                        nc.vector._custom_dve(
                            _MIN2, out=sc[:], in0=pt[:, 0:half], in1=cp[:],
                            s0=3.0e38, accum_out=acc)

            nc.sync.dma_start(out=outT, in_=mins[:])

    nc.compile()
    return nc


def _get_program(n_rt):
    key = (n_rt, MM_DT, CHUNK_OF_GROUP, REDUCER)
    if key not in _CACHE:
        _CACHE[key] = _build(n_rt)
    return _CACHE[key]


def _transform(points, poses, idx):
    P = poses[idx]                                   # [N,4,4]
    R, t = P[:, :3, :3], P[:, :3, 3]
    return np.einsum('nij,nj->ni', R, points) + t    # [N,3]


def kernel(points, time_indice, est_poses, gt_poses):
    points = np.asarray(points, dtype=np.float32)
    ti = np.asarray(time_indice)
    est_poses = np.asarray(est_poses, dtype=np.float32)
    gt_poses = np.asarray(gt_poses, dtype=np.float32)

    est = _transform(points, est_poses, ti)          # [N,3]
    gt = _transform(points, gt_poses, ti)            # [N,3]
    est_sq = np.sum(est * est, axis=1)               # [N]
    gt_sq = np.sum(gt * gt, axis=1)                  # [N]

    sel = np.flatnonzero(ti == 1)
    m = sel.size
    denom = np.float32(m) + np.float32(1e-7)
    if m == 0:
        return np.float32(0.0), np.float32(0.0)

    l2 = np.float32(
        np.linalg.norm((est[sel] - gt[sel]).astype(np.float64), axis=1).sum()
        / denom)

    n_rt = -(-m // 128)
    mpad = n_rt * 128
    pad = np.concatenate([sel, np.repeat(sel[:1], mpad - m)])

    def lhs_for(sel_pts):
        out = np.empty((4, mpad), np.float32)
        out[:3] = (-2.0 * sel_pts[pad]).T
        out[3] = 1.0
        return out

    def rhs_for(pts, sq, c, k):
        s = slice(c * NSHARD + k * FREE, c * NSHARD + (k + 1) * FREE)
        out = np.empty((4, FREE), np.float32)
        out[:3] = pts[s].T
        out[3] = sq[s]
        return out

    lhsA = lhs_for(gt)    # dist1: selected gt rows vs all est points
    lhsB = lhs_for(est)   # dist2: selected est rows vs all gt points
    in_maps = [
        {
            f"inp{g}": np.concatenate(
                [lhsA, lhsB,
                 rhs_for(est, est_sq, c, CHUNK_OF_GROUP[g]),
                 rhs_for(gt, gt_sq, c, CHUNK_OF_GROUP[g])], axis=1)
            for g in range(4)
        }
        for c in range(N_CORES)
    ]

    nc = _get_program(n_rt)
    results = run_bass_kernel_spmd(nc, in_maps, list(range(N_CORES))).results

    # [128, 2*n_rt] per core -> global min across cores -> flatten row-tiles
    allout = np.min([r["out"] for r in results], axis=0)
    partA = allout[:, :n_rt].T.ravel()[:m]
    partB = allout[:, n_rt:].T.ravel()[:m]
    dist1 = partA.astype(np.float64) + gt_sq[sel]
    dist2 = partB.astype(np.float64) + est_sq[sel]
    chamfer = np.float32(0.5 * (dist1.sum() + dist2.sum()) / denom)
    return chamfer, l2


# revision 10
# speedup vs baseline: 1.3380x; 1.3380x over previous
"""Chamfer-distance kernel for TRN2 (8 NeuronCores, SPMD).

Math: the reference weights w are nonzero ONLY for points with
time_indice == 1 (m of N points).  So of the NxN distance matrix we only
need row-mins for the m selected rows (dist1) and col-mins for the m
selected columns (dist2) -- each an (m x N) problem, min over N.

Each (m x N) pass is computed as a K=4 matmul:
    C[i, j] = sq[j] - 2 * dot(sel_i, pts_j)
with lhsT rows 0..2 = -2*sel coords, row 3 = ones, and rhs rows 0..2 =
pts coords, row 3 = |pts|^2.  The per-row constant sq[i] of the selected
point is added on the host after the global min.

Perf structure (per 128-row tile, 2048 columns on each core):
  * the 4 512-col chunk matmuls are packed into 4 distinct PE row-groups
    via tile_position (K=4 only occupies 4 of 128 PE rows) and run
    concurrently; the two chunks of the PSUM hi half are issued FIRST so
    the Scalar-engine copy can start at the half-tile point;
  * PSUM is one 4-bank [128, 2048] tile (double-buffered); the Scalar
    engine copies the hi half to SBUF while the Vector engine runs a
    tensor_tensor_reduce (min, min) that folds the PSUM lo half against
    the copy while row-min-reducing -- 2 PSUM elements drained per DVE
    cycle, which is the architectural ceiling (DVE and ACT have one
    32-bit PSUM read port each; GPSIMD has none, DMA is far slower).
  * All inputs for one PE row-group (lhsA | lhsB | rhs chunks) are packed
    in ONE dram tensor so the whole input load is 4 DMA_DIRECT2D issues
    split over both HWDGE queues (the per-issue cost is ~750ns, so the
    baseline's 16 input DMAs wasted ~5us of startup).

Sharding: the N search points are split 2048-per-core across 8 cores
(same lhsT everywhere); each core returns per-row partial mins, the host
takes the elementwise min across cores and does the tiny O(m) tail.
"""

import numpy as np

import concourse.bass as bass
import concourse.mybir as mybir
import concourse.tile as tile
from concourse import bacc
from concourse import dve_ops as _dvo
from concourse.bass_utils import run_bass_kernel_spmd
from concourse.dve_spec import Spec, Src0, Src1, C0, AluOp, minn, lower
from concourse.dve_spec import _has_src1 as _has_src1
from concourse.dve_uop import DveOpSpec


def _make_min2():
    """Register a custom DVE op: out = min(in0, in1), accum_out = row-min."""
    name = "MIN2_REDUCE_ANT"
    for o in _dvo.OPS:
        if o.name == name:
            return o

    def _ref(in0, in1, s0, s1, imm2):
        b = np.minimum(in0, in1).astype(np.float32)
        seed = np.asarray(s0, np.float32).reshape(-1, 1)
        acc = np.minimum(b.reshape(b.shape[0], -1).min(axis=-1, keepdims=True), seed)
        return b, acc

    spec = Spec(body=minn(Src0, Src1), accum=AluOp.MIN, accum_init=C0,
                reference=_ref)
    op = _dvo.DveOp(name, spec, subdim=False, uops_sha={})
    _dvo.OPS.append(op)
    _dvo.CUSTOM_DVE_SPECS[name] = spec
    _dvo._SUB_OPCODE_FOR_NAME[name] = _dvo._CUSTOM_DVE_ROW_BASE + len(_dvo.OPS) - 1
    for ver in ("v3", "v4"):
        ds = DveOpSpec(name=name, opcode=_dvo.get_dve_sub_opcode(name),
                       uops=lower(spec, ver=ver), rd1_en=_has_src1(spec))
        op.uops_sha[ver] = ds.sha(ver)
    return op


_MIN2 = _make_min2()

N_CORES = 8
N_POINTS = 16384
NSHARD = N_POINTS // N_CORES  # 2048 search points per core
FREE = 512                    # matmul moving free dim (one PSUM bank of fp32)
NCC = NSHARD // FREE          # 4 column chunks per row tile

# dtype used for the matmul operands: float32r streams faster than fp32's
# LOW_HIGH dual pass at slightly reduced internal precision.
MM_DT = "float32r"
REDUCER = "min2"   # "min2" = custom DVE op, "ttr" = builtin tensor_tensor_reduce
# PE row-group g computes column chunk CHUNK_OF_GROUP[g].  The hi-half
# chunks (2, 3) go to groups 0/1 so their input DMAs land first and the
# scalar-engine copy of the hi half can start at the half-tile point.
CHUNK_OF_GROUP = (2, 3, 0, 1)

_CACHE = {}


EARLY = 256   # pass-A lhs columns shipped in the early DMA (2 row tiles)


def _build(n_rt):
    """Build + compile the SPMD Bass program for n_rt row-tiles of 128."""
    f32 = mybir.dt.float32
    mdt = getattr(mybir.dt, MM_DT)
    mpad = n_rt * 128
    half = NSHARD // 2
    # Two dram tensors per PE row-group.  "early" carries the rhs chunks
    # plus the first EARLY lhs columns so compute starts ~2us after the
    # queues come up; "rest" streams the remaining weights during compute
    # (weight demand is ~1.6 GB/s, far under the ~9 GB/s a 4-partition DMA
    # sustains, so the stream stays ahead of the consumer).
    wE = 2 * FREE + EARLY
    wR = 2 * mpad - EARLY

    nc = bacc.Bacc("TRN2", target_bir_lowering=False, debug=False,
                   num_devices=N_CORES, enable_partition_id=False)
    earlies = [nc.dram_tensor(f"early{g}", [4, wE], mdt, kind="ExternalInput").ap()
               for g in range(4)]
    rests = [nc.dram_tensor(f"rest{g}", [4, wR], mdt, kind="ExternalInput").ap()
             for g in range(4)]
    outT = nc.dram_tensor("out", [128, 2 * n_rt], f32, kind="ExternalOutput").ap()

    with tile.TileContext(nc) as tc:
        with (
            tc.tile_pool(name="inp", bufs=1) as inp,
            tc.tile_pool(name="res", bufs=1) as res,
            tc.tile_pool(name="cpy", bufs=3) as cpy,
            tc.tile_pool(name="scr", bufs=2) as scr,
            tc.tile_pool(name="ps", bufs=2, space="PSUM") as ps,
        ):
            # Row-group g's data lives on partitions 32g..32g+3.  Early DMAs
            # for all 4 groups issue first (alternating HWDGE queues), then
            # the big weight streams.
            itE = inp.tile([128, wE], mdt, tag="itE")
            itR = inp.tile([128, wR], mdt, tag="itR")
            for g in range(4):
                q = nc.sync if g % 2 == 0 else nc.scalar
                q.dma_start(out=itE[32 * g:32 * g + 4, :], in_=earlies[g])
            for g in range(4):
                q = nc.sync if g % 2 == 0 else nc.scalar
                q.dma_start(out=itR[32 * g:32 * g + 4, :], in_=rests[g])

            def lhs_slice(g, p, rt):
                """Weight columns for (pass p, row tile rt) on group g."""
                gp = slice(32 * g, 32 * g + 4)
                c0 = p * mpad + 128 * rt
                if c0 + 128 <= EARLY:
                    return itE[gp, 2 * FREE + c0:2 * FREE + c0 + 128]
                return itR[gp, c0 - EARLY:c0 - EARLY + 128]

            mins = res.tile([128, 2 * n_rt], f32, tag="mins")

            for p in range(2):  # 0 = pass A (dist1), 1 = pass B (dist2)
                for rt in range(n_rt):
                    lo = ps.tile([128, half], f32, tag="lo")
                    hi = ps.tile([128, half], f32, tag="hi")
                    for g in range(4):
                        cc = CHUNK_OF_GROUP[g]
                        gp = slice(32 * g, 32 * g + 4)
                        dst = (hi if cc >= 2 else lo)[:, bass.ts(cc % 2, FREE)]
                        nc.tensor.matmul(
                            dst,
                            lhs_slice(g, p, rt),
                            itE[gp, p * FREE:(p + 1) * FREE],
                            start=True, stop=True,
                            tile_position=(32 * g, 0),
                        )
                    # split the row-min: ACT copies the hi half to SBUF, DVE
                    # min-combines the lo PSUM half with it while reducing.
                    cp = cpy.tile([128, half], f32, tag="cp")
                    nc.scalar.copy(out=cp[:], in_=hi[:])
                    sc = scr.tile([128, half], f32, tag="sc")
                    acc = mins[:, p * n_rt + rt:p * n_rt + rt + 1]
                    if REDUCER == "ttr":
                        nc.vector.tensor_tensor_reduce(
                            out=sc[:], in0=lo[:], in1=cp[:],
                            scale=1.0, scalar=3.0e38,
                            op0=mybir.AluOpType.min, op1=mybir.AluOpType.min,
                            accum_out=acc,
                        )
                    else:
                        nc.vector._custom_dve(
                            _MIN2, out=sc[:], in0=lo[:], in1=cp[:],
                            s0=3.0e38, accum_out=acc)

            nc.sync.dma_start(out=outT, in_=mins[:])

    nc.compile()
    return nc


def _get_program(n_rt):
    key = (n_rt, MM_DT, CHUNK_OF_GROUP, REDUCER, EARLY)
    if key not in _CACHE:
        _CACHE[key] = _build(n_rt)
    return _CACHE[key]


def _transform(points, poses, idx):
    P = poses[idx]                                   # [N,4,4]
    R, t = P[:, :3, :3], P[:, :3, 3]
    return np.einsum('nij,nj->ni', R, points) + t    # [N,3]


def kernel(points, time_indice, est_poses, gt_poses):
    points = np.asarray(points, dtype=np.float32)
    ti = np.asarray(time_indice)
    est_poses = np.asarray(est_poses, dtype=np.float32)
    gt_poses = np.asarray(gt_poses, dtype=np.float32)

    est = _transform(points, est_poses, ti)          # [N,3]
    gt = _transform(points, gt_poses, ti)            # [N,3]
    est_sq = np.sum(est * est, axis=1)               # [N]
    gt_sq = np.sum(gt * gt, axis=1)                  # [N]

    sel = np.flatnonzero(ti == 1)
    m = sel.size
    denom = np.float32(m) + np.float32(1e-7)
    if m == 0:
        return np.float32(0.0), np.float32(0.0)

    l2 = np.float32(
        np.linalg.norm((est[sel] - gt[sel]).astype(np.float64), axis=1).sum()
        / denom)

    n_rt = -(-m // 128)
    mpad = n_rt * 128
    pad = np.concatenate([sel, np.repeat(sel[:1], mpad - m)])

    def lhs_for(sel_pts):
        out = np.empty((4, mpad), np.float32)
        out[:3] = (-2.0 * sel_pts[pad]).T
        out[3] = 1.0
        return out

    def rhs_for(pts, sq, c, k):
        s = slice(c * NSHARD + k * FREE, c * NSHARD + (k + 1) * FREE)
        out = np.empty((4, FREE), np.float32)
        out[:3] = pts[s].T
        out[3] = sq[s]
        return out

    lhsA = lhs_for(gt)    # dist1: selected gt rows vs all est points
    lhsB = lhs_for(est)   # dist2: selected est rows vs all gt points
    L = np.concatenate([lhsA, lhsB], axis=1)       # [4, 2*mpad]
    in_maps = []
    for c in range(N_CORES):
        im = {}
        for g in range(4):
            cc = CHUNK_OF_GROUP[g]
            im[f"early{g}"] = np.concatenate(
                [rhs_for(est, est_sq, c, cc),
                 rhs_for(gt, gt_sq, c, cc),
                 L[:, :EARLY]], axis=1)
            im[f"rest{g}"] = np.ascontiguousarray(L[:, EARLY:])
        in_maps.append(im)

    nc = _get_program(n_rt)
    results = run_bass_kernel_spmd(nc, in_maps, list(range(N_CORES))).results

    # [128, 2*n_rt] per core -> global min across cores -> flatten row-tiles
    allout = np.min([r["out"] for r in results], axis=0)
    partA = allout[:, :n_rt].T.ravel()[:m]
    partB = allout[:, n_rt:].T.ravel()[:m]
    dist1 = partA.astype(np.float64) + gt_sq[sel]
    dist2 = partB.astype(np.float64) + est_sq[sel]
    chamfer = np.float32(0.5 * (dist1.sum() + dist2.sum()) / denom)
    return chamfer, l2


# revision 13
# speedup vs baseline: 1.3927x; 1.0409x over previous
"""Chamfer-distance kernel for TRN2 (8 NeuronCores, SPMD).

Math: the reference weights w are nonzero ONLY for points with
time_indice == 1 (m of N points).  So of the NxN distance matrix we only
need row-mins for the m selected rows (dist1) and col-mins for the m
selected columns (dist2) -- each an (m x N) problem, min over N.

Each (m x N) pass is computed as a K=4 matmul:
    C[i, j] = sq[j] - 2 * dot(sel_i, pts_j)
with lhsT rows 0..2 = -2*sel coords, row 3 = ones, and rhs rows 0..2 =
pts coords, row 3 = |pts|^2.  The per-row constant sq[i] of the selected
point is added on the host after the global min.

Perf structure (per 128-row tile, 2048 columns on each core):
  * the 4 512-col chunk matmuls are packed into 4 distinct PE row-groups
    via tile_position (K=4 only occupies 4 of 128 PE rows) and run
    concurrently; the two chunks of the PSUM hi half are issued FIRST so
    the Scalar-engine copy can start at the half-tile point;
  * PSUM is one 4-bank [128, 2048] tile (double-buffered); the Scalar
    engine copies the hi half to SBUF while the Vector engine runs a
    tensor_tensor_reduce (min, min) that folds the PSUM lo half against
    the copy while row-min-reducing -- 2 PSUM elements drained per DVE
    cycle, which is the architectural ceiling (DVE and ACT have one
    32-bit PSUM read port each; GPSIMD has none, DMA is far slower).
  * All inputs for one PE row-group (lhsA | lhsB | rhs chunks) are packed
    in ONE dram tensor so the whole input load is 4 DMA_DIRECT2D issues
    split over both HWDGE queues (the per-issue cost is ~750ns, so the
    baseline's 16 input DMAs wasted ~5us of startup).

Sharding: the N search points are split 2048-per-core across 8 cores
(same lhsT everywhere); each core returns per-row partial mins, the host
takes the elementwise min across cores and does the tiny O(m) tail.
"""

import numpy as np

import concourse.bass as bass
import concourse.mybir as mybir
import concourse.tile as tile
from concourse import bacc
from concourse import dve_ops as _dvo
from concourse.bass_utils import run_bass_kernel_spmd
from concourse.dve_spec import Spec, Src0, Src1, C0, AluOp, minn, lower
from concourse.dve_spec import _has_src1 as _has_src1
from concourse.dve_uop import DveOpSpec


def _make_min2():
    """Register a custom DVE op: out = min(in0, in1), accum_out = row-min."""
    name = "MIN2_REDUCE_ANT"
    for o in _dvo.OPS:
        if o.name == name:
            return o

    def _ref(in0, in1, s0, s1, imm2):
        b = np.minimum(in0, in1).astype(np.float32)
        seed = np.asarray(s0, np.float32).reshape(-1, 1)
        acc = np.minimum(b.reshape(b.shape[0], -1).min(axis=-1, keepdims=True), seed)
        return b, acc

    spec = Spec(body=minn(Src0, Src1), accum=AluOp.MIN, accum_init=C0,
                reference=_ref)
    op = _dvo.DveOp(name, spec, subdim=False, uops_sha={})
    _dvo.OPS.append(op)
    _dvo.CUSTOM_DVE_SPECS[name] = spec
    _dvo._SUB_OPCODE_FOR_NAME[name] = _dvo._CUSTOM_DVE_ROW_BASE + len(_dvo.OPS) - 1
    for ver in ("v3", "v4"):
        ds = DveOpSpec(name=name, opcode=_dvo.get_dve_sub_opcode(name),
                       uops=lower(spec, ver=ver), rd1_en=_has_src1(spec))
        op.uops_sha[ver] = ds.sha(ver)
    return op


_MIN2 = _make_min2()

N_CORES = 8
N_POINTS = 16384
NSHARD = N_POINTS // N_CORES  # 2048 search points per core
FREE = 512                    # matmul moving free dim (one PSUM bank of fp32)
NCC = NSHARD // FREE          # 4 column chunks per row tile

# dtype used for the matmul operands: float32r streams faster than fp32's
# LOW_HIGH dual pass at slightly reduced internal precision.
MM_DT = "float32r"
REDUCER = "min2"   # "min2" = custom DVE op, "ttr" = builtin tensor_tensor_reduce
# PE row-group g computes column chunk CHUNK_OF_GROUP[g].  The hi-half
# chunks (2, 3) go to groups 0/1 so their input DMAs land first and the
# scalar-engine copy of the hi half can start at the half-tile point.
CHUNK_OF_GROUP = (2, 3, 0, 1)

_CACHE = {}


EARLY = 256   # pass-A lhs columns shipped with the first rhs DMA (2 row tiles)


def _build(n_rt):
    """Build + compile the SPMD Bass program for n_rt row-tiles of 128."""
    f32 = mybir.dt.float32
    mdt = getattr(mybir.dt, MM_DT)
    mpad = n_rt * 128
    half = NSHARD // 2
    # Four dram tensors per PE row-group, sized so the first compute
    # dependencies ship first (DMA bandwidth is ~2.2 GB/s per partition
    # row and transfers on one HWDGE queue serialize):
    #   eA: rhsA chunk + first EARLY lhs cols  -> pass A starts ~2us in
    #   eB: rhsB chunk                         -> needed from tile n_rt on
    #   rA: rest of pass-A lhs; rB: pass-B lhs -> stream under compute
    wEA = FREE + EARLY
    wRA = mpad - EARLY

    nc = bacc.Bacc("TRN2", target_bir_lowering=False, debug=False,
                   num_devices=N_CORES, enable_partition_id=False)
    eA = [nc.dram_tensor(f"eA{g}", [4, wEA], mdt, kind="ExternalInput").ap()
          for g in range(4)]
    eB = [nc.dram_tensor(f"eB{g}", [4, FREE], mdt, kind="ExternalInput").ap()
          for g in range(4)]
    rA = [nc.dram_tensor(f"rA{g}", [4, wRA], mdt, kind="ExternalInput").ap()
          for g in range(4)]
    rB = [nc.dram_tensor(f"rB{g}", [4, mpad], mdt, kind="ExternalInput").ap()
          for g in range(4)]
    outA = nc.dram_tensor("outA", [128, n_rt], f32, kind="ExternalOutput").ap()
    outB = nc.dram_tensor("outB", [128, n_rt], f32, kind="ExternalOutput").ap()

    with tile.TileContext(nc) as tc:
        with (
            tc.tile_pool(name="inp", bufs=1) as inp,
            tc.tile_pool(name="res", bufs=1) as res,
            tc.tile_pool(name="cpy", bufs=3) as cpy,
            tc.tile_pool(name="scr", bufs=2) as scr,
            tc.tile_pool(name="ps", bufs=2, space="PSUM") as ps,
        ):
            # Row-group g's data lives on partitions 32g..32g+3.  Issue
            # order per HWDGE queue = dependency order of the pipeline.
            tEA = inp.tile([128, wEA], mdt, tag="tEA")
            tEB = inp.tile([128, FREE], mdt, tag="tEB")
            tRA = inp.tile([128, wRA], mdt, tag="tRA")
            tRB = inp.tile([128, mpad], mdt, tag="tRB")
            for src, dst in ((eA, tEA), (eB, tEB), (rA, tRA), (rB, tRB)):
                for g in range(4):
                    q = nc.sync if g % 2 == 0 else nc.scalar
                    q.dma_start(out=dst[32 * g:32 * g + 4, :], in_=src[g])

            def lhs_slice(g, p, rt):
                """Weight columns for (pass p, row tile rt) on group g."""
                gp = slice(32 * g, 32 * g + 4)
                c0 = 128 * rt
                if p == 1:
                    return tRB[gp, c0:c0 + 128]
                if c0 + 128 <= EARLY:
                    return tEA[gp, FREE + c0:FREE + c0 + 128]
                return tRA[gp, c0 - EARLY:c0 - EARLY + 128]

            minsA = res.tile([128, n_rt], f32, tag="minsA")
            minsB = res.tile([128, n_rt], f32, tag="minsB")

            for p in range(2):  # 0 = pass A (dist1), 1 = pass B (dist2)
                mins = minsA if p == 0 else minsB
                rhs = tEA if p == 0 else tEB
                for rt in range(n_rt):
                    lo = ps.tile([128, half], f32, tag="lo")
                    hi = ps.tile([128, half], f32, tag="hi")
                    for g in range(4):
                        cc = CHUNK_OF_GROUP[g]
                        gp = slice(32 * g, 32 * g + 4)
                        dst = (hi if cc >= 2 else lo)[:, bass.ts(cc % 2, FREE)]
                        nc.tensor.matmul(
                            dst,
                            lhs_slice(g, p, rt),
                            rhs[gp, 0:FREE],
                            start=True, stop=True,
                            tile_position=(32 * g, 0),
                        )
                    # split the row-min: ACT copies the hi half to SBUF, DVE
                    # min-combines the lo PSUM half with it while reducing.
                    cp = cpy.tile([128, half], f32, tag="cp")
                    nc.scalar.copy(out=cp[:], in_=hi[:])
                    sc = scr.tile([128, half], f32, tag="sc")
                    acc = mins[:, rt:rt + 1]
                    if REDUCER == "ttr":
                        nc.vector.tensor_tensor_reduce(
                            out=sc[:], in0=lo[:], in1=cp[:],
                            scale=1.0, scalar=3.0e38,
                            op0=mybir.AluOpType.min, op1=mybir.AluOpType.min,
                            accum_out=acc,
                        )
                    else:
                        nc.vector._custom_dve(
                            _MIN2, out=sc[:], in0=lo[:], in1=cp[:],
                            s0=3.0e38, accum_out=acc)
                # ship pass A's result while pass B computes
                nc.sync.dma_start(out=outA if p == 0 else outB, in_=mins[:])

    nc.compile()
    return nc


def _get_program(n_rt):
    key = (n_rt, MM_DT, CHUNK_OF_GROUP, REDUCER, EARLY)
    if key not in _CACHE:
        _CACHE[key] = _build(n_rt)
    return _CACHE[key]


def _transform(points, poses, idx):
    P = poses[idx]                                   # [N,4,4]
    R, t = P[:, :3, :3], P[:, :3, 3]
    return np.einsum('nij,nj->ni', R, points) + t    # [N,3]


def kernel(points, time_indice, est_poses, gt_poses):
    points = np.asarray(points, dtype=np.float32)
    ti = np.asarray(time_indice)
    est_poses = np.asarray(est_poses, dtype=np.float32)
    gt_poses = np.asarray(gt_poses, dtype=np.float32)

    est = _transform(points, est_poses, ti)          # [N,3]
    gt = _transform(points, gt_poses, ti)            # [N,3]
    est_sq = np.sum(est * est, axis=1)               # [N]
    gt_sq = np.sum(gt * gt, axis=1)                  # [N]

    sel = np.flatnonzero(ti == 1)
    m = sel.size
    denom = np.float32(m) + np.float32(1e-7)
    if m == 0:
        return np.float32(0.0), np.float32(0.0)

    l2 = np.float32(
        np.linalg.norm((est[sel] - gt[sel]).astype(np.float64), axis=1).sum()
        / denom)

    n_rt = -(-m // 128)
    mpad = n_rt * 128
    pad = np.concatenate([sel, np.repeat(sel[:1], mpad - m)])

    def lhs_for(sel_pts):
        out = np.empty((4, mpad), np.float32)
        out[:3] = (-2.0 * sel_pts[pad]).T
        out[3] = 1.0
        return out

    def rhs_for(pts, sq, c, k):
        s = slice(c * NSHARD + k * FREE, c * NSHARD + (k + 1) * FREE)
        out = np.empty((4, FREE), np.float32)
        out[:3] = pts[s].T
        out[3] = sq[s]
        return out

    lhsA = lhs_for(gt)    # dist1: selected gt rows vs all est points
    lhsB = lhs_for(est)   # dist2: selected est rows vs all gt points
    in_maps = []
    for c in range(N_CORES):
        im = {}
        for g in range(4):
            cc = CHUNK_OF_GROUP[g]
            im[f"eA{g}"] = np.concatenate(
                [rhs_for(est, est_sq, c, cc), lhsA[:, :EARLY]], axis=1)
            im[f"eB{g}"] = rhs_for(gt, gt_sq, c, cc)
            im[f"rA{g}"] = np.ascontiguousarray(lhsA[:, EARLY:])
            im[f"rB{g}"] = lhsB
        in_maps.append(im)

    nc = _get_program(n_rt)
    results = run_bass_kernel_spmd(nc, in_maps, list(range(N_CORES))).results

    # [128, n_rt] per core -> global min across cores -> flatten row-tiles
    partA = np.min([r["outA"] for r in results], axis=0).T.ravel()[:m]
    partB = np.min([r["outB"] for r in results], axis=0).T.ravel()[:m]
    dist1 = partA.astype(np.float64) + gt_sq[sel]
    dist2 = partB.astype(np.float64) + est_sq[sel]
    chamfer = np.float32(0.5 * (dist1.sum() + dist2.sum()) / denom)
    return chamfer, l2
